# revision 3
# baseline (speedup 1.0000x reference)
"""Multi-head attention (B=4, S=2048, D=1024, H=16, causal) on 8 trn2 cores.

Sharding: data-parallel over batch (4) x tensor-parallel over head groups (2).
Core c handles batch b=c//2, heads g=c%2 (8 heads each). Each core computes
its partial output projection into fp16; host sums the two partials per batch
and adds the bias.

Single fused pipeline per core (fp16 matmul inputs, fp32 accumulation):
  - projections run in 4 column-batches (512 s-cols of qT/kT for all head
    pairs + 4 t-blocks of v) interleaved with attention units of the s-blocks
    each batch enables, so the DVE-paced softmax stream starts early and the
    PE-paced projections hide under it.
  - per unit (s-block i, head): negscores [128, W] in PSUM 1024-chunks
    (Wq negated on host), fused mask-add + row-min on DVE straight into an
    SBUF fp32 scratch (PSUM bank frees right after), one exp per unit on ACT
    (scale=-1, bias=row-min) -> E fp16, E -> E^T via DMA-transpose XBAR (SP
    queue), PV with lhsT=E^T tile, rhs=[v_tile | 1] -> out[s, dk+1]; the
    65th column accumulates the softmax denominator.
  - concat[s, hd] = PV * (1/den) (DVE recip, Pool scales), DMA-transpose ->
    concat^T, output projection (K=512) -> PSUM -> fp16 (Pool) -> DRAM via
    Pool SWDGE.
"""

import math

import numpy as np

B, S, D, H = 4, 2048, 1024, 16
DK = 64
HLOC = 8          # heads per core
HD = HLOC * DK    # 512 local concat dims
P = 128
NBLK = S // P     # 16 s-blocks
KO = D // P       # 8 contraction tiles for projections
MPAIRS = 4        # head pairs per core
NEG = 30000.0     # mask addend on negated scores
CH = 512          # score psum chunk width (1 bank)
LAG = 11          # units between scores emission and PV emission


def build():
    import concourse.bass as bass  # noqa: F401
    import concourse.mybir as mybir
    import concourse.tile as tile
    from concourse import bacc

    fp16 = mybir.dt.float16
    f32 = mybir.dt.float32
    Alu = mybir.AluOpType
    Ax = mybir.AxisListType

    nc = bacc.Bacc()

    xtq = nc.dram_tensor("xtq", [D, S], fp16, kind="ExternalInput")
    xtk = nc.dram_tensor("xtk", [D, S], fp16, kind="ExternalInput")
    xtv = nc.dram_tensor("xtv", [D, S], fp16, kind="ExternalInput")
    wq = nc.dram_tensor("wq", [D, HD], fp16, kind="ExternalInput")
    wk = nc.dram_tensor("wk", [D, HD], fp16, kind="ExternalInput")
    wv = nc.dram_tensor("wv", [D, HD], fp16, kind="ExternalInput")
    wo = nc.dram_tensor("wo", [HD, D], fp16, kind="ExternalInput")
    masktri = nc.dram_tensor("masktri", [P, P], f32, kind="ExternalInput")
    y = nc.dram_tensor("y", [S, D], fp16, kind="ExternalOutput")

    with tile.TileContext(nc) as tc:
        with (
            tc.tile_pool(name="persist", bufs=1) as persist,
            tc.tile_pool(name="stats", bufs=32) as stats,
            tc.tile_pool(name="xt", bufs=6) as xtpool,
            tc.tile_pool(name="chunks", bufs=5, space="PSUM") as chunkpool,
            tc.tile_pool(name="yp", bufs=1, space="PSUM") as ypool,
            tc.tile_pool(name="pvp", bufs=1, space="PSUM") as pvpool,
            tc.tile_pool(name="sc32", bufs=3) as sc32pool,
            tc.tile_pool(name="ebuf", bufs=4) as ebufpool,
            tc.tile_pool(name="pts", bufs=LAG + 2) as ptspool,
            tc.tile_pool(name="csb", bufs=2) as cpool,
            tc.tile_pool(name="ct", bufs=2) as ctpool,
            tc.tile_pool(name="ysb", bufs=2) as ysbpool,
        ):
            wq_sb = persist.tile([P, KO, HD], fp16, tag="wq", name="wq")
            wk_sb = persist.tile([P, KO, HD], fp16, tag="wk", name="wk")
            wv_sb = persist.tile([P, KO, HD], fp16, tag="wv", name="wv")
            nc.sync.dma_start(out=wk_sb, in_=wk[:].rearrange("(ko p) n -> p ko n", p=P))
            nc.sync.dma_start(out=wq_sb, in_=wq[:].rearrange("(ko p) n -> p ko n", p=P))
            mtri_sb = persist.tile([P, P], f32, tag="mtri", name="mtri")
            nc.sync.dma_start(out=mtri_sb, in_=masktri[:])
            nc.sync.dma_start(out=wv_sb, in_=wv[:].rearrange("(ko p) n -> p ko n", p=P))
            wo_sb = persist.tile([P, MPAIRS, D], fp16, tag="wo", name="wo")
            nc.sync.dma_start(out=wo_sb, in_=wo[:].rearrange("(m p) n -> p m n", p=P))

            qt = persist.tile([P, MPAIRS, S], fp16, tag="qt", name="qt")  # hd%128
            kt = persist.tile([P, MPAIRS, S], fp16, tag="kt", name="kt")
            # v + ones column: [t%128, t//128, h, dk|1]
            vv = persist.tile([P, NBLK, HLOC, DK + 1], fp16, tag="vv", name="vv")
            nc.gpsimd.memset(vv[:, :, :, DK : DK + 1], 1.0)

            xq_r = xtq[:].rearrange("(ko p) s -> p ko s", p=P)
            xk_r = xtk[:].rearrange("(ko p) s -> p ko s", p=P)
            xv_r = xtv[:].rearrange("(ko p) s -> p ko s", p=P)

            NB = 8      # projection batches of 256 s-columns / 2 t-blocks
            BW = S // NB  # 256

            def load_batch(c):
                """Load the c-th 256-column slab of X^T for q, k, v."""
                tiles = []
                for src_r in (xq_r, xk_r, xv_r):
                    xsb = xtpool.tile([P, KO, BW], fp16, tag="xt", name="xt")
                    nc.sync.dma_start(
                        out=xsb, in_=src_r[:, :, c * BW : (c + 1) * BW]
                    )
                    tiles.append(xsb)
                return tiles

            def proj_groups(c, xq_sb, xk_sb, xv_sb):
                """Return per-group closures projecting columns
                [BW*c, BW*(c+1)) of qT/kT and t-blocks 2c, 2c+1 of v."""
                groups = []
                for wsb, xsb, dst, eng in (
                    (wq_sb, xq_sb, qt, nc.scalar),
                    (wk_sb, xk_sb, kt, nc.gpsimd),
                ):
                    for m in range(MPAIRS):
                        def g(wsb=wsb, xsb=xsb, dst=dst, eng=eng, m=m):
                            ps = pspool.tile([P, 512], f32, tag="ps", name="ps")
                            for ko in range(KO):
                                nc.tensor.matmul(
                                    ps[:, 0:BW],
                                    lhsT=wsb[:, ko, m * P : (m + 1) * P],
                                    rhs=xsb[:, ko, :],
                                    start=(ko == 0),
                                    stop=(ko == KO - 1),
                                )
                            if eng is nc.scalar:
                                eng.copy(
                                    out=dst[:, m, c * BW : (c + 1) * BW],
                                    in_=ps[:, 0:BW],
                                )
                            else:
                                eng.tensor_copy(
                                    out=dst[:, m, c * BW : (c + 1) * BW],
                                    in_=ps[:, 0:BW],
                                )
                        groups.append(g)
                for tq in range(2):
                    tm = 2 * c + tq
                    def g(tm=tm, tq=tq, xv_sb=xv_sb):
                        ps = pspool.tile([P, 512], f32, tag="ps", name="ps")
                        for ko in range(KO):
                            nc.tensor.matmul(
                                ps[:, 0:512],
                                lhsT=xv_sb[:, ko, tq * P : (tq + 1) * P],
                                rhs=wv_sb[:, ko, :],
                                start=(ko == 0),
                                stop=(ko == KO - 1),
                            )
                        nc.gpsimd.tensor_copy(
                            out=vv[:, tm, :, 0:DK],
                            in_=ps[:, 0:512].rearrange("p (h k) -> p h k", h=HLOC),
                        )
                    groups.append(g)
                return groups

            # unit schedule: per batch c, the two enabled s-blocks (bigger
            # first); units of each block iterate (m, z)
            units = []
            for c in range(NB):
                for i in (2 * c + 1, 2 * c):
                    for m in range(MPAIRS):
                        for z in (0, 1):
                            units.append((i, m, z))
            NU = len(units)  # 128
            UPB = NU // NB   # 16 units per batch

            state = {}
            pv_tiles = {}
            csb_tiles = {}
            ct_tiles = {}
            rden_tiles = {}
            pending = {}

            def emit_unit(u):
                i, m, z = units[u]
                W = (i + 1) * P
                off = z * DK
                nch = (W + CH - 1) // CH
                sc32 = sc32pool.tile([P, S], f32, tag="sc32", name="sc32")
                nmx = stats.tile([P, 4], f32, tag="nmx", name="nmx")
                for cc in range(nch):
                    cw = min(CH, W - cc * CH)
                    c0 = cc * CH
                    ck = chunkpool.tile([P, CH], f32, tag="ck", name="ck")
                    for h2 in range(0, cw, 512):
                        hw = min(512, cw - h2)
                        nc.tensor.matmul(
                            ck[:, h2 : h2 + hw],
                            lhsT=qt[off : off + DK, m, i * P : (i + 1) * P],
                            rhs=kt[off : off + DK, m, c0 + h2 : c0 + h2 + hw],
                            start=True,
                            stop=True,
                        )
                    # fused causal-mask add + running row-min (of negated
                    # scores) in one DVE pass, landing in SBUF so the PSUM
                    # bank frees immediately; maskw slides so its +NEG
                    # triangle lands on the diagonal block
                    moff = MASKW - W + c0
                    nc.vector.tensor_tensor_reduce(
                        out=sc32[:, c0 : c0 + cw],
                        in0=ck[:, 0:cw],
                        in1=maskw_sb[:, moff : moff + cw],
                        scale=1.0,
                        scalar=3.0e38,
                        op0=Alu.add,
                        op1=Alu.min,
                        accum_out=nmx[:, cc : cc + 1],
                    )
                if nch > 1:
                    negmx = stats.tile([P, 1], f32, tag="negmx", name="negmx")
                    nc.vector.tensor_reduce(
                        negmx, nmx[:, 0:nch], axis=Ax.X, op=Alu.min
                    )
                else:
                    negmx = nmx[:, 0:1]
                ebuf = ebufpool.tile([P, S], fp16, tag="ebuf", name="ebuf")
                nc.scalar.activation(
                    out=ebuf[:, 0:W],
                    in_=sc32[:, 0:W],
                    func=mybir.ActivationFunctionType.Exp,
                    bias=negmx,
                    scale=-1.0,
                )
                pts = ptspool.tile([P, NBLK, P], fp16, tag="pts", name="pts")
                nc.sync.dma_start(
                    out=pts[:, 0 : i + 1, :], in_=ebuf[:, 0:W], transpose=True
                )
                state[u] = pts

            def emit_pv(u):
                i, m, z = units[u]
                pts = state.pop(u)
                if m == 0 and z == 0:
                    pv_tiles[i] = [
                        pvpool.tile([P, MPAIRS, DK + 1], f32, tag=f"pv{zz}",
                                    name=f"pv{zz}")
                        for zz in (0, 1)
                    ]
                pv = pv_tiles[i][z]
                for j in range(i + 1):
                    nc.tensor.matmul(
                        pv[:, m, :],
                        lhsT=pts[:, j, :],
                        rhs=vv[:, j, 2 * m + z, :],
                        start=(j == 0),
                        stop=(j == i),
                    )

            def emit_recip(i, m, z):
                if m == 0 and z == 0:
                    csb_tiles[i] = cpool.tile(
                        [P, HLOC, DK], fp16, tag="csb", name="csb"
                    )
                pv = pv_tiles[i][z]
                rden = stats.tile([P, 1], f32, tag="rden", name="rden")
                rden_tiles[(i, m, z)] = rden
                nc.vector.reciprocal(rden, pv[:, m, DK : DK + 1])

            def emit_scale(i, m, z):
                csb = csb_tiles[i]
                pv = pv_tiles[i][z]
                rden = rden_tiles.pop((i, m, z))
                if z == 0:
                    nc.vector.tensor_scalar_mul(
                        csb[:, 2 * m + z, :], pv[:, m, 0:DK], rden
                    )
                else:
                    nc.scalar.activation(
                        out=csb[:, 2 * m + z, :], in_=pv[:, m, 0:DK],
                        func=mybir.ActivationFunctionType.Copy, scale=rden,
                    )

            def emit_ct(i):
                csb = csb_tiles.pop(i)
                ct = ctpool.tile([P, MPAIRS, P], fp16, tag="ct", name="ct")
                ct_tiles[i] = ct
                nc.sync.dma_start(
                    out=ct, in_=csb[:].rearrange("p h k -> p (h k)"),
                    transpose=True,
                )

            def emit_outproj(i):
                ct = ct_tiles.pop(i)
                ysb = ysbpool.tile([P, D], fp16, tag="ysb", name="ysb")
                for nch in range(2):
                    yps = pspool.tile([P, 512], f32, tag="ps", name="ps")
                    for kk in range(MPAIRS):
                        nc.tensor.matmul(
                            yps[:, 0:512],
                            lhsT=ct[:, kk, :],
                            rhs=wo_sb[:, kk, nch * 512 : (nch + 1) * 512],
                            start=(kk == 0),
                            stop=(kk == MPAIRS - 1),
                        )
                    nc.gpsimd.tensor_copy(
                        out=ysb[:, nch * 512 : (nch + 1) * 512], in_=yps[:, 0:512]
                    )
                nc.gpsimd.dma_start(out=y[:][i * P : (i + 1) * P, :], in_=ysb)

            # ---- main interleaved emission loop ----
            # batch 0 projections run as a dense prologue; batch c+1's
            # projection groups are spread across batch c's units
            for g in proj_groups(0, *load_batch(0)):
                g()
            gq = []
            for u in range(NU + LAG + 8):
                for fn in pending.pop(u, ()):
                    fn()
                if u < NU and u % UPB == 0 and u // UPB + 1 < NB:
                    gq.extend(proj_groups(u // UPB + 1, *load_batch(u // UPB + 1)))
                    spread = max(1, UPB // len(gq))
                if u < NU and gq and u % spread == spread - 1:
                    gq.pop(0)()
                if u < NU:
                    emit_unit(u)
                w = u - LAG
                if 0 <= w < NU:
                    emit_pv(w)
                    i, m, z = units[w]
                    if m == MPAIRS - 1:
                        pending.setdefault(u + 1, []).append(
                            lambda i=i, z=z: emit_recip(i, z)
                        )
                        pending.setdefault(u + 2, []).append(
                            lambda i=i, z=z: emit_scale(i, z)
                        )
                        if z == 1:
                            pending.setdefault(u + 4, []).append(
                                lambda i=i: emit_ct(i)
                            )
                            pending.setdefault(u + 6, []).append(
                                lambda i=i: emit_outproj(i)
                            )
                while u >= NU and gq:
                    gq.pop(0)()
            for fns in [pending[k] for k in sorted(pending)]:
                for fn in fns:
                    fn()
            for ctx in (ptss_ctx, ptsb_ctx, ebuf_ctx):
                ctx.__exit__(None, None, None)

    nc.finalize()
    return nc


def _prep_inputs(Q, K, V, Wq, Wk, Wv, Wo):
    """Host-side shard + layout prep. Returns list of 8 in_maps."""
    rt8 = math.sqrt(math.sqrt(64.0))  # sqrt(8): scale split over q and k
    in_maps = []
    # diagonal-block causal mask: -NEG above the diagonal
    tri = np.where(
        np.arange(P)[:, None] < np.arange(P)[None, :], np.float32(-NEG), 0.0
    ).astype(np.float32)
    for c in range(8):
        b, g = c // 2, c % 2
        heads = slice(g * HLOC, (g + 1) * HLOC)
        # [H,D,DK] -> [D, HLOC*DK]; q negated so row-max becomes row-min
        wq_p = (Wq[heads] * rt8).transpose(1, 0, 2).reshape(D, HD)
        wk_p = (Wk[heads] * rt8).transpose(1, 0, 2).reshape(D, HD)
        wv_p = Wv[heads].transpose(1, 0, 2).reshape(D, HD)
        wo_p = Wo[:, g * HD : (g + 1) * HD].T  # [HD, D]
        in_maps.append({
            "xtq": np.ascontiguousarray(Q[b].T).astype(np.float16),
            "xtk": np.ascontiguousarray(K[b].T).astype(np.float16),
            "xtv": np.ascontiguousarray(V[b].T).astype(np.float16),
            "wq": np.ascontiguousarray(wq_p).astype(np.float16),
            "wk": np.ascontiguousarray(wk_p).astype(np.float16),
            "wv": np.ascontiguousarray(wv_p).astype(np.float16),
            "wo": np.ascontiguousarray(wo_p).astype(np.float16),
            "masktri": tri,
        })
    return in_maps


_NC = []


def kernel(Q, K, V, mask, Wq, Wk, Wv, Wo, bo, _trace=False):
    from concourse.bass_utils import run_bass_kernel_spmd

    Q, K, V = np.asarray(Q), np.asarray(K), np.asarray(V)
    Wq, Wk, Wv = np.asarray(Wq), np.asarray(Wk), np.asarray(Wv)
    Wo, bo = np.asarray(Wo), np.asarray(bo)

    if not _NC:
        _NC.append(build())
    nc = _NC[0]
    in_maps = _prep_inputs(Q, K, V, Wq, Wk, Wv, Wo)
    res = run_bass_kernel_spmd(nc, in_maps, core_ids=list(range(8)), trace=_trace)
    ys = [r["y"].astype(np.float32) for r in res.results]
    out = np.stack([ys[2 * b] + ys[2 * b + 1] for b in range(B)])
    out = out + bo[None, None, :].astype(np.float32)
    if _trace:
        kernel._last = res
    return out.astype(np.float32)
